# revision 22
# baseline (speedup 1.0000x reference)
"""Trainium2 Bass kernel for nn_MultiHeadAttention (B=2, C=1024, H=16, S=2048).

Sharding: 8 cores = 2 batches x 4 head-groups (4 heads per core).

The kernel is ACT(exp)-paced: 128 exps of [128, 1024] PSUM->SBUF (~1.0us
back-to-back) =~ 128us floor; everything else is scheduled to hide under it.

Structure (per core):
  - 8 phases (p, blk): head-pair p x 512-query block blk, ordered
    (0,0),(1,0),(0,1),(1,1),... so the odd phases (pair 1) need no new DMA
    and the input stream hides behind them.
  - Per body (one j-tile): ONE sc tile [128, 2(hh), 512] (2 PSUM banks),
    2 score matmuls that co-issue on disjoint PE row groups (partition
    halves 0:64/64:128), ONE exp of FD=1024.
  - exp writes into a pt PAIR tile [128, 2(j), 2(hh), 512]; the mask
    multiply runs once per (pair, hh) as an FD=1024 bf16 2x DVE op with a
    strided [2, 512] access pattern over two j-rows of maskT.
  - ctx matmuls for a j-pair are emitted 3 bodies later (after the mask is
    guaranteed done) so the PE FIFO never head-of-line blocks ahead of the
    next scores. ctx uses the 65-col ones trick for the softmax denominator.
  - PSUM: 2 score slots (2 banks each) + 2 ctx accumulators (1 bank each)
    + a DEDICATED 2-slot projection pool (1 bank each) = 8 banks. The
    projection chunks never share slots with scores, so their PSUM->SBUF
    bias moves (DVE, deferred one body) are fully elastic.
  - DMA: one instruction per 512-column group via a (x p) n rearrange
    (8 DRAM row-blocks per instruction), masks two j-rows at a time;
    need-ordered so phase (0,0) is gated only by W+K+V+Q[0:512]+mask-q0.
  - Host does the final divide by the denominator row + transpose/concat.
"""

import numpy as np
import ml_dtypes

import concourse.bass as bass
import concourse.mybir as mybir
import concourse.tile as tile
from concourse import bacc
from concourse.bass_utils import run_bass_kernel_spmd

B = 2
C = 1024
HEADS = 16
CPH = 64
S = 2048
N_CORES = 8
HPC = 4  # heads per core
CPC = HPC * CPH  # channels per core = 256

BF = mybir.dt.bfloat16
F32 = mybir.dt.float32
EXP = mybir.ActivationFunctionType.Exp

NBF = ml_dtypes.bfloat16

_NC_CACHE = {}

# phase order: (p, blk)
PHASES = [(0, 0), (1, 0), (0, 1), (1, 1), (0, 2), (1, 2), (0, 3), (1, 3)]


def build_nc():
    nc = bacc.Bacc("TRN2", target_bir_lowering=False)

    Qd = nc.declare_dram_parameter("Qin", [C, S], BF, isOutput=False)
    Kd = nc.declare_dram_parameter("Kin", [C, S], BF, isOutput=False)
    Vd = nc.declare_dram_parameter("Vin", [C, S], BF, isOutput=False)
    WqTd = nc.declare_dram_parameter("WqT", [128, 8 * CPC], BF, isOutput=False)
    WkTd = nc.declare_dram_parameter("WkT", [128, 8 * CPC], BF, isOutput=False)
    WvTd = nc.declare_dram_parameter("WvT", [128, 8 * HPC * 65], BF, isOutput=False)
    bqkd = nc.declare_dram_parameter("bqk", [128, 4], F32, isOutput=False)
    bvbd = nc.declare_dram_parameter("bvb", [128, HPC * 65], F32, isOutput=False)
    Md = nc.declare_dram_parameter("maskT", [S, S], BF, isOutput=False)
    Od = nc.declare_dram_parameter("out", [HPC * 65, S], BF, isOutput=True)

    with tile.TileContext(nc) as tc:
        with (
            tc.tile_pool(name="w", bufs=1) as wp,
            tc.tile_pool(name="qksb", bufs=1) as qkp,
            tc.tile_pool(name="vt", bufs=1) as vtp,
            tc.tile_pool(name="msk", bufs=1) as mkp,
            tc.tile_pool(name="ioqk", bufs=1) as ioqk,
            tc.tile_pool(name="pt", bufs=3) as ptp,
            tc.tile_pool(name="ob", bufs=2) as obp,
            tc.tile_pool(name="sc", bufs=2, space="PSUM") as scp,
            tc.tile_pool(name="cx", bufs=2, space="PSUM") as cxp,
            tc.tile_pool(name="prj", bufs=2, space="PSUM") as prjp,
        ):
            # --- persistent SBUF tensors ---
            WqT = wp.tile([128, 8, CPC], BF, tag="wq")
            WkT = wp.tile([128, 8, CPC], BF, tag="wk")
            WvT = wp.tile([128, 8, HPC * 65], BF, tag="wv")
            bqk = wp.tile([128, 4], F32, tag="bqk")
            bvb = wp.tile([128, HPC * 65], F32, tag="bvb")
            dummy = wp.tile([128, 1], F32, tag="dum")
            dummy2 = wp.tile([128, 128], BF, tag="dum2")  # warmup lhsT/rhs

            q_sb = qkp.tile([128, 2, S], BF, tag="q")  # pair-major, head rows 0:64/64:128
            k_sb = qkp.tile([128, 2, S], BF, tag="k")
            vT = vtp.tile([128, 16, HPC * 65], BF, tag="vt")  # s_tile-major
            maskT = mkp.tile([128, 16, S], BF, tag="m")
            Qin = ioqk.tile([128, 8, S], BF, tag="qi")
            Kin = ioqk.tile([128, 8, S], BF, tag="ki")
            # V is consumed early (phase (0,0)); half-size buffer, h1 is
            # re-DMA'd over h0 via the idle GPSIMD queue (no head-of-line
            # blocking of the main sync DMA stream)
            Vin = ioqk.tile([128, 8, 1024], BF, tag="vi")

            # --- DMA emitters; emission order sets arrival order ---
            def dma_cols(buf, dram, c0, w):
                # one instruction: all 8 row-blocks of a column group
                nc.sync.dma_start(
                    buf[:, :, bass.ds(c0, w)],
                    dram[:, bass.ds(c0, w)].rearrange("(x p) n -> p x n", x=8),
                )

            def dma_m2(j2, c0, w):
                # two j-rows (256 DRAM rows) per instruction
                nc.sync.dma_start(
                    maskT[:, 2 * j2 : 2 * j2 + 2, bass.ds(c0, w)],
                    Md[bass.ds(256 * j2, 256), bass.ds(c0, w)].rearrange(
                        "(x p) n -> p x n", x=2
                    ),
                )

            # weights first (needed by the upfront projections)
            for wt, wd in ((WvT, WvTd), (WkT, WkTd), (WqT, WqTd)):
                nc.sync.dma_start(wt[:], wd[:].rearrange("p (t n) -> p t n", t=8))
            nc.sync.dma_start(bqk[:], bqkd[:])
            nc.sync.dma_start(bvb[:], bvbd[:])
            # absorb the exp ACT_TABLE_LOAD (~2.7us) during startup
            nc.scalar.activation(dummy[:], bqk[:, 0:1], EXP)

            # need-ordered input stream. Phase (0,0) is gated by:
            # W + K(full) + V(full) + Q[0:512] + mask q0 ~ 12.7 MB.
            dma_cols(Kin, Kd, 0, 512)
            dma_cols(Qin, Qd, 0, 512)
            dma_cols(Vin, Vd, 0, 512)
            for j2 in range(2):
                dma_m2(j2, 0, 512)
            dma_cols(Kin, Kd, 512, 512)
            for j2 in range(2, 4):
                dma_m2(j2, 0, 512)
            dma_cols(Vin, Vd, 512, 512)
            for j2 in range(4, 6):
                dma_m2(j2, 0, 512)
            dma_cols(Kin, Kd, 1024, 512)
            dma_m2(6, 0, 512)
            dma_cols(Kin, Kd, 1536, 512)
            dma_m2(7, 0, 512)
            dma_cols(Qin, Qd, 512, 512)
            for j2 in range(8):  # mask q1 (for phases blk=1)
                dma_m2(j2, 512, 512)
            dma_cols(Qin, Qd, 1024, 512)
            for j2 in range(8):  # mask h1 (for phases blk=2,3)
                dma_m2(j2, 1024, 1024)
            dma_cols(Qin, Qd, 1536, 512)

            # --- projection chunks: own 1-bank PSUM slots, deferred bias ---
            def proj_v(s):
                ps = prjp.tile([128, 512], F32, tag="prj", name="pv")
                for ci in range(8):
                    nc.tensor.matmul(
                        ps[:, : HPC * 65],
                        lhsT=Vin[:, ci, bass.ts(s % 8, 128)],
                        rhs=WvT[:, ci, :],
                        start=(ci == 0),
                        stop=(ci == 7),
                    )

                def bias():
                    nc.vector.tensor_add(vT[:, s, :], ps[:, : HPC * 65], bvb[:])

                return bias

            def proj_qk(p, qk, n4):
                dst, wt, src = (
                    (q_sb, WqT, Qin) if qk == 0 else (k_sb, WkT, Kin)
                )
                ps = prjp.tile([128, 512], F32, tag="prj", name="pqk")
                for ci in range(8):
                    nc.tensor.matmul(
                        ps[:],
                        lhsT=wt[:, ci, bass.ts(p, 128)],
                        rhs=src[:, ci, bass.ts(n4, 512)],
                        start=(ci == 0),
                        stop=(ci == 7),
                    )

                def bias():
                    nc.vector.tensor_scalar_add(
                        dst[:, p, bass.ts(n4, 512)],
                        ps[:],
                        bqk[:, 2 * p + qk : 2 * p + qk + 1],
                    )

                return bias

            # PE warmup: ~8us of garbage matmuls gated on nothing so the
            # HAM clock-gate reaches 8/8 before the first projection; they
            # span the DMA window and cost no useful time
            wps = prjp.tile([128, 512], F32, tag="prj", name="warm")
            nc.vector.memset(dummy2[:], 0.0)
            for _ in range(48):
                nc.tensor.matmul(
                    wps[:, :128],
                    lhsT=dummy2[:],
                    rhs=dummy2[:],
                    start=True,
                    stop=True,
                )

            # upfront projections: only what scores_0 needs — everything
            # else goes into loop bodies behind the scores on the PE FIFO
            proj_qk(0, 1, 0)()
            proj_qk(0, 0, 0)()

            def refill_v(half):
                # V h1 re-DMA over the consumed h0 half-buffer; GPSIMD
                # queue so the WAR wait can't head-of-line block the main
                # input stream. Emitted AFTER the last h0 readers so the
                # dependency is visible to the tile framework.
                nc.gpsimd.dma_start(
                    Vin[:, :, bass.ts(half, 512)],
                    Vd[:, bass.ds(1024 + half * 512, 512)].rearrange(
                        "(x p) n -> p x n", x=8
                    ),
                )

            # in-loop projection schedule: phase index -> {body j: [chunks]}
            il = {
                0: {  # (0,0) — DMA-gated phase; PE slack absorbs the chunks
                    0: [lambda: proj_v(0), lambda: proj_v(1)],
                    1: [lambda: proj_v(2), lambda: proj_qk(0, 1, 1)],
                    2: [lambda: proj_v(3), lambda: proj_v(4)],
                    3: [lambda: proj_v(5)],
                    5: [lambda: proj_qk(0, 1, 2)],
                    6: [lambda: proj_v(6)],
                    7: [lambda: proj_v(7)],
                    9: [lambda: proj_v(8), lambda: proj_v(9)],
                    10: [lambda: proj_qk(0, 1, 3)],
                    12: [lambda: proj_v(10), lambda: proj_v(11)],
                    13: [lambda: proj_v(12), lambda: proj_qk(1, 1, 0)],
                    14: [lambda: proj_v(13), lambda: proj_qk(1, 0, 0)],
                    15: [lambda: proj_v(14), lambda: proj_v(15)],
                },
                1: {  # (1,0)
                    1: [lambda: proj_qk(1, 1, 1)],
                    5: [lambda: proj_qk(1, 1, 2)],
                    9: [lambda: proj_qk(1, 1, 3)],
                    12: [lambda: proj_qk(0, 0, 1)],
                },
                2: {  # (0,1)
                    2: [lambda: proj_qk(1, 0, 1)],
                    6: [lambda: proj_qk(0, 0, 2)],
                },
                3: {2: [lambda: proj_qk(1, 0, 2)]},  # (1,1)
                4: {2: [lambda: proj_qk(0, 0, 3)]},  # (0,2)
                5: {2: [lambda: proj_qk(1, 0, 3)]},  # (1,2)
                6: {},  # (0,3)
                7: {},  # (1,3)
            }

            # --- attention: flat 128-body stream with an event schedule ---
            # events[g] = closures to emit at global body g (after scores+exp)
            events = {}

            def add_event(g, fn, front=False):
                lst = events.setdefault(g, [])
                if front:
                    lst.insert(0, fn)
                else:
                    lst.append(fn)

            cx_of = {}  # phase index -> [cx_hh0, cx_hh1]
            pt_of = {}  # (phase index, pair m) -> pt pair tile

            def mk_masks(ph, m):
                p, blk = PHASES[ph]

                def fn():
                    pt = pt_of.pop((ph, m))
                    for hh in range(2):
                        nc.vector.tensor_mul(
                            pt[:, :, hh, :],
                            pt[:, :, hh, :],
                            maskT[:, 2 * m : 2 * m + 2, bass.ds(blk * 512, 512)],
                        )
                    pt_of[(ph, m, "masked")] = pt

                return fn

            def mk_ctx(ph, m, jp):
                # one j-tile's ctx (2 MMs) per body — splitting the 4-MM
                # pair event in half keeps the PE FIFO load uniform so the
                # scores->exp chain never overruns the ACT pace
                p, blk = PHASES[ph]

                def fn():
                    key = (ph, m, "masked")
                    pt = pt_of[key]
                    if jp == 1:
                        del pt_of[key]
                    cxt = cx_of[ph]
                    jj = 2 * m + jp
                    for hh in range(2):
                        hloc = 2 * p + hh
                        nc.tensor.matmul(
                            cxt[hh][:],
                            lhsT=vT[:, jj, bass.ds(hloc * 65, 65)],
                            rhs=pt[:, jp, hh, :],
                            start=(jj == 0),
                            stop=(jj == 15),
                        )

                return fn

            def mk_drain(ph):
                p, blk = PHASES[ph]

                def fn():
                    cxt = cx_of.pop(ph)
                    for hh in range(2):
                        hloc = 2 * p + hh
                        ob = obp.tile([65, 512], BF, tag="ob")
                        nc.vector.tensor_copy(ob[:], cxt[hh][:])
                        nc.sync.dma_start(
                            Od[bass.ds(hloc * 65, 65), bass.ds(blk * 512, 512)],
                            ob[:],
                        )

                return fn

            for ph in range(8):
                base = 16 * ph
                for m in range(8):
                    add_event(base + 2 * m + 2, mk_masks(ph, m))
                    add_event(base + 2 * m + 3, mk_ctx(ph, m, 0))
                    add_event(base + 2 * m + 4, mk_ctx(ph, m, 1))
                # drain one body after the last ctx so its DVE copies don't
                # sit ahead of the NEXT phase's first mask ops; at the list
                # FRONT so the next phase's ctx (same body) still sees the
                # WAR on the cx banks
                add_event(base + 19, mk_drain(ph), front=True)

            pend_bias = []
            for g in range(16 * 8 + 4):
                if g < 16 * 8:
                    ph = g // 16
                    p, blk = PHASES[ph]
                    j = g % 16
                    if j == 3:
                        # allocated after the previous phase's drain (emitted
                        # at body base+2) so the WAR dependency is visible
                        cx_of[ph] = [
                            cxp.tile([65, 512], F32, tag="cx", name=f"cx{ph}{i}")
                            for i in range(2)
                        ]
                    # deferred DVE bias moves from the previous body's chunks
                    for fn in pend_bias:
                        fn()
                    pend_bias = []
                    # scores: one sc tile, two co-issued matmuls, one exp
                    sc = scp.tile([128, 2, 512], F32, tag="sc")
                    for hh in range(2):
                        lo, hi = 64 * hh, 64 * hh + 64
                        nc.tensor.matmul(
                            sc[:, hh, :],
                            lhsT=k_sb[lo:hi, p, bass.ts(j, 128)],
                            rhs=q_sb[lo:hi, p, bass.ts(blk, 512)],
                            start=True,
                            stop=True,
                        )
                    if j % 2 == 0:
                        ptpair = ptp.tile([128, 2, 2, 512], BF, tag="pt")
                        pt_of[(ph, j // 2)] = ptpair
                    else:
                        ptpair = pt_of[(ph, j // 2)]
                    nc.scalar.activation(ptpair[:, j % 2, :, :], sc[:], EXP)
                # scheduled events (masks / lagged ctx / drains)
                for fn in events.pop(g, ()):
                    fn()
                # projection chunks last on the PE FIFO
                if g < 16 * 8:
                    if ph == 0 and j == 4:
                        refill_v(0)
                    elif ph == 0 and j == 8:
                        refill_v(1)
                    for mk in il[ph].get(j, ()):
                        pend_bias.append(mk())
            for fn in pend_bias:
                fn()
    nc.compile()
    return nc


def _get_nc():
    if "nc" not in _NC_CACHE:
        _NC_CACHE["nc"] = build_nc()
    return _NC_CACHE["nc"]


def _make_in_maps(Q, K, V, mask, Wq, bq, Wk, bk, Wv, bv):
    per_batch = []
    for b in range(B):
        Qa = Q[b].astype(NBF)
        Ka = K[b].astype(NBF)
        Va = V[b].astype(NBF)
        mT = np.ascontiguousarray((~mask[b]).T).astype(np.float32).astype(NBF)
        per_batch.append((Qa, Ka, Va, mT))

    in_maps = []
    for c in range(N_CORES):
        b, g = divmod(c, 4)
        hs = slice(g * CPC, (g + 1) * CPC)
        Qa, Ka, Va, mT = per_batch[b]
        # pre-arranged to the SBUF layout [128, ci, n] so the weight DMA
        # is a single contiguous transfer
        WqTa = np.ascontiguousarray(
            (Wq[hs].T / 8.0).reshape(8, 128, CPC).transpose(1, 0, 2).reshape(128, -1)
        ).astype(NBF)
        WkTa = np.ascontiguousarray(
            Wk[hs].T.reshape(8, 128, CPC).transpose(1, 0, 2).reshape(128, -1)
        ).astype(NBF)
        WvTa = np.zeros((C, HPC * 65), np.float32)
        bvba = np.zeros((128, HPC * 65), np.float32)
        for hh in range(HPC):
            ch = slice((g * HPC + hh) * CPH, (g * HPC + hh + 1) * CPH)
            WvTa[:, hh * 65 : hh * 65 + 64] = Wv[ch].T
            bvba[:, hh * 65 : hh * 65 + 64] = bv[ch][None, :]
            bvba[:, hh * 65 + 64] = 1.0
        # bias for q/k psum->sbuf copies: col 2p+qk = per-partition bias of
        # pair p's 128 channels (rows 0:64 = head 2p, 64:128 = head 2p+1)
        bqka = np.zeros((128, 4), np.float32)
        for p in range(2):
            ch = slice((g * 2 + p) * 128, (g * 2 + p + 1) * 128)
            bqka[:, 2 * p] = bq[ch] / 8.0
            bqka[:, 2 * p + 1] = bk[ch]
        in_maps.append(
            {
                "Qin": Qa,
                "Kin": Ka,
                "Vin": Va,
                "WqT": WqTa,
                "WkT": WkTa,
                "WvT": np.ascontiguousarray(
                    WvTa.reshape(8, 128, HPC * 65)
                    .transpose(1, 0, 2)
                    .reshape(128, -1)
                ).astype(NBF),
                "bqk": bqka,
                "bvb": bvba,
                "maskT": mT,
            }
        )
    return in_maps


def _assemble(results):
    out = np.zeros((B, S, C), np.float32)
    for c in range(N_CORES):
        b, g = divmod(c, 4)
        o = results[c]["out"].astype(np.float32)  # [260, 2048]
        for hh in range(HPC):
            ctx = o[hh * 65 : hh * 65 + 64]  # [64, S] = (d, i)
            den = o[hh * 65 + 64]  # [S]
            ch0 = (g * HPC + hh) * CPH
            out[b, :, ch0 : ch0 + CPH] = (ctx / den[None, :]).T
    return out


def run(inputs, trace=False):
    in_maps = _make_in_maps(
        np.asarray(inputs["Q"], np.float32),
        np.asarray(inputs["K"], np.float32),
        np.asarray(inputs["V"], np.float32),
        np.asarray(inputs["mask"]),
        np.asarray(inputs["Wq"], np.float32),
        np.asarray(inputs["bq"], np.float32),
        np.asarray(inputs["Wk"], np.float32),
        np.asarray(inputs["bk"], np.float32),
        np.asarray(inputs["Wv"], np.float32),
        np.asarray(inputs["bv"], np.float32),
    )
    br = run_bass_kernel_spmd(_get_nc(), in_maps, list(range(N_CORES)), trace=trace)
    return _assemble(br.results), br


def kernel(**inputs) -> np.ndarray:
    out, _ = run(inputs)
    return out


# revision 30
# speedup vs baseline: 1.0444x; 1.0444x over previous
"""Trainium2 Bass kernel for nn_MultiHeadAttention (B=2, C=1024, H=16, S=2048).

Sharding: 8 cores = 2 batches x 4 head-groups (4 heads per core).

The kernel is ACT(exp)-paced: 128 exps of [128, 1024] PSUM->SBUF (~1.0us
back-to-back) =~ 128us floor; everything else is scheduled to hide under it.

Structure (per core):
  - 8 phases (p, blk): head-pair p x 512-query block blk, ordered
    (0,0),(1,0),(0,1),(1,1),... so the odd phases (pair 1) need no new DMA
    and the input stream hides behind them.
  - Per body (one j-tile): ONE sc tile [128, 2(hh), 512] (2 PSUM banks),
    2 score matmuls that co-issue on disjoint PE row groups (partition
    halves 0:64/64:128), ONE exp of FD=1024.
  - exp writes into a pt PAIR tile [128, 2(j), 2(hh), 512]; the mask
    multiply runs once per (pair, hh) as an FD=1024 bf16 2x DVE op with a
    strided [2, 512] access pattern over two j-rows of maskT.
  - ctx matmuls for a j-pair are emitted 3 bodies later (after the mask is
    guaranteed done) so the PE FIFO never head-of-line blocks ahead of the
    next scores. ctx uses the 65-col ones trick for the softmax denominator.
  - PSUM: 2 score slots (2 banks each) + 2 ctx accumulators (1 bank each)
    + a DEDICATED 2-slot projection pool (1 bank each) = 8 banks. The
    projection chunks never share slots with scores, so their PSUM->SBUF
    bias moves (DVE, deferred one body) are fully elastic.
  - DMA: one instruction per 512-column group via a (x p) n rearrange
    (8 DRAM row-blocks per instruction), masks two j-rows at a time;
    need-ordered so phase (0,0) is gated only by W+K+V+Q[0:512]+mask-q0.
  - Host does the final divide by the denominator row + transpose/concat.
"""

import numpy as np
import ml_dtypes

import concourse.bass as bass
import concourse.mybir as mybir
import concourse.tile as tile
from concourse import bacc
from concourse.bass_utils import run_bass_kernel_spmd

B = 2
C = 1024
HEADS = 16
CPH = 64
S = 2048
N_CORES = 8
HPC = 4  # heads per core
CPC = HPC * CPH  # channels per core = 256

BF = mybir.dt.bfloat16
F32 = mybir.dt.float32
EXP = mybir.ActivationFunctionType.Exp

NBF = ml_dtypes.bfloat16

_NC_CACHE = {}

# phase order: (p, blk)
PHASES = [(0, 0), (1, 0), (0, 1), (1, 1), (0, 2), (1, 2), (0, 3), (1, 3)]


def build_nc():
    nc = bacc.Bacc("TRN2", target_bir_lowering=False)

    Qd = nc.declare_dram_parameter("Qin", [C, S], BF, isOutput=False)
    Kd = nc.declare_dram_parameter("Kin", [C, S], BF, isOutput=False)
    Vd = nc.declare_dram_parameter("Vin", [C, S], BF, isOutput=False)
    WqTd = nc.declare_dram_parameter("WqT", [128, 8 * CPC], BF, isOutput=False)
    WkTd = nc.declare_dram_parameter("WkT", [128, 8 * CPC], BF, isOutput=False)
    WvTd = nc.declare_dram_parameter("WvT", [128, 8 * HPC * 65], BF, isOutput=False)
    bqkd = nc.declare_dram_parameter("bqk", [128, 4], F32, isOutput=False)
    bvbd = nc.declare_dram_parameter("bvb", [128, HPC * 65], F32, isOutput=False)
    Md = nc.declare_dram_parameter("maskT", [S, S], BF, isOutput=False)
    Od = nc.declare_dram_parameter("out", [HPC * 65, S], BF, isOutput=True)

    with tile.TileContext(nc) as tc:
        with (
            tc.tile_pool(name="w", bufs=1) as wp,
            tc.tile_pool(name="qksb", bufs=1) as qkp,
            tc.tile_pool(name="vt", bufs=1) as vtp,
            tc.tile_pool(name="msk", bufs=1) as mkp,
            tc.tile_pool(name="ioqk", bufs=1) as ioqk,
            tc.tile_pool(name="pt", bufs=3) as ptp,
            tc.tile_pool(name="ob", bufs=2) as obp,
            tc.tile_pool(name="sc", bufs=2, space="PSUM") as scp,
            tc.tile_pool(name="cx", bufs=2, space="PSUM") as cxp,
            tc.tile_pool(name="prj", bufs=2, space="PSUM") as prjp,
        ):
            # --- persistent SBUF tensors ---
            WqT = wp.tile([128, 8, CPC], BF, tag="wq")
            WkT = wp.tile([128, 8, CPC], BF, tag="wk")
            WvT = wp.tile([128, 8, HPC * 65], BF, tag="wv")
            bqk = wp.tile([128, 4], F32, tag="bqk")
            bvb = wp.tile([128, HPC * 65], F32, tag="bvb")
            dummy = wp.tile([128, 1], F32, tag="dum")

            q_sb = qkp.tile([128, 2, S], BF, tag="q")  # pair-major, head rows 0:64/64:128
            k_sb = qkp.tile([128, 2, S], BF, tag="k")
            vT = vtp.tile([128, 16, HPC * 65], BF, tag="vt")  # s_tile-major
            maskT = mkp.tile([128, 16, S], BF, tag="m")
            Qin = ioqk.tile([128, 8, S], BF, tag="qi")
            Kin = ioqk.tile([128, 8, S], BF, tag="ki")
            # V is consumed early (phase (0,0)); half-size buffer, h1 is
            # re-DMA'd over h0 via the idle GPSIMD queue (no head-of-line
            # blocking of the main sync DMA stream)
            Vin = ioqk.tile([128, 8, 1024], BF, tag="vi")

            # --- DMA emitters; emission order sets arrival order ---
            def dma_cols(buf, dram, c0, w):
                # one instruction: all 8 row-blocks of a column group
                nc.sync.dma_start(
                    buf[:, :, bass.ds(c0, w)],
                    dram[:, bass.ds(c0, w)].rearrange("(x p) n -> p x n", x=8),
                )

            def dma_m2(j2, c0, w):
                # two j-rows (256 DRAM rows) per instruction
                nc.sync.dma_start(
                    maskT[:, 2 * j2 : 2 * j2 + 2, bass.ds(c0, w)],
                    Md[bass.ds(256 * j2, 256), bass.ds(c0, w)].rearrange(
                        "(x p) n -> p x n", x=2
                    ),
                )

            # weights first (needed by the upfront projections)
            for wt, wd in ((WvT, WvTd), (WkT, WkTd), (WqT, WqTd)):
                nc.sync.dma_start(wt[:], wd[:].rearrange("p (t n) -> p t n", t=8))
            nc.sync.dma_start(bqk[:], bqkd[:])
            nc.sync.dma_start(bvb[:], bvbd[:])
            # absorb the exp ACT_TABLE_LOAD (~2.7us) during startup
            nc.scalar.activation(dummy[:], bqk[:, 0:1], EXP)

            # need-ordered input stream. Phase (0,0) is gated by:
            # W + K(full) + V(full) + Q[0:512] + mask q0 ~ 12.7 MB.
            dma_cols(Kin, Kd, 0, 512)
            dma_cols(Qin, Qd, 0, 512)
            dma_cols(Vin, Vd, 0, 512)
            for j2 in range(2):
                dma_m2(j2, 0, 512)
            dma_cols(Kin, Kd, 512, 512)
            for j2 in range(2, 4):
                dma_m2(j2, 0, 512)
            dma_cols(Vin, Vd, 512, 512)
            for j2 in range(4, 6):
                dma_m2(j2, 0, 512)
            dma_cols(Kin, Kd, 1024, 512)
            dma_m2(6, 0, 512)
            dma_cols(Kin, Kd, 1536, 512)
            dma_m2(7, 0, 512)
            dma_cols(Qin, Qd, 512, 512)
            for j2 in range(8):  # mask q1 (for phases blk=1)
                dma_m2(j2, 512, 512)
            dma_cols(Qin, Qd, 1024, 512)
            for j2 in range(8):  # mask h1 (for phases blk=2,3)
                dma_m2(j2, 1024, 1024)
            dma_cols(Qin, Qd, 1536, 512)

            # --- projection chunks: own 1-bank PSUM slots, deferred bias ---
            def proj_v(s):
                ps = prjp.tile([128, 512], F32, tag="prj", name="pv")
                for ci in range(8):
                    nc.tensor.matmul(
                        ps[:, : HPC * 65],
                        lhsT=Vin[:, ci, bass.ts(s % 8, 128)],
                        rhs=WvT[:, ci, :],
                        start=(ci == 0),
                        stop=(ci == 7),
                    )

                def bias():
                    nc.vector.tensor_add(vT[:, s, :], ps[:, : HPC * 65], bvb[:])

                return bias

            def proj_qk(p, qk, n4):
                dst, wt, src = (
                    (q_sb, WqT, Qin) if qk == 0 else (k_sb, WkT, Kin)
                )
                ps = prjp.tile([128, 512], F32, tag="prj", name="pqk")
                for ci in range(8):
                    nc.tensor.matmul(
                        ps[:],
                        lhsT=wt[:, ci, bass.ts(p, 128)],
                        rhs=src[:, ci, bass.ts(n4, 512)],
                        start=(ci == 0),
                        stop=(ci == 7),
                    )

                def bias():
                    nc.vector.tensor_scalar_add(
                        dst[:, p, bass.ts(n4, 512)],
                        ps[:],
                        bqk[:, 2 * p + qk : 2 * p + qk + 1],
                    )

                return bias

            # upfront projections: only what scores_0 needs — everything
            # else goes into loop bodies behind the scores on the PE FIFO
            proj_qk(0, 1, 0)()
            proj_qk(0, 0, 0)()

            def refill_v(half):
                # V h1 re-DMA over the consumed h0 half-buffer; GPSIMD
                # queue so the WAR wait can't head-of-line block the main
                # input stream. Emitted AFTER the last h0 readers so the
                # dependency is visible to the tile framework.
                nc.gpsimd.dma_start(
                    Vin[:, :, bass.ts(half, 512)],
                    Vd[:, bass.ds(1024 + half * 512, 512)].rearrange(
                        "(x p) n -> p x n", x=8
                    ),
                )

            # in-loop projection schedule: phase index -> {body j: [chunks]}
            il = {
                0: {  # (0,0) — DMA-gated phase; PE slack absorbs the chunks
                    0: [("v", 0), ("v", 1)],
                    1: [("v", 2), ("qk", 0, 1, 1)],
                    2: [("v", 3), ("v", 4)],
                    3: [("v", 5)],
                    5: [("qk", 0, 1, 2)],
                    6: [("v", 6)],
                    7: [("v", 7)],
                    9: [("v", 8), ("v", 9)],
                    10: [("qk", 0, 1, 3)],
                    11: [("v", 10)],
                    12: [("v", 11)],
                    13: [("v", 12), ("qk", 1, 1, 0)],
                    14: [("v", 13), ("qk", 1, 0, 0)],
                    15: [("v", 14), ("v", 15)],
                },
                1: {  # (1,0)
                    1: [("qk", 1, 1, 1)],
                    5: [("qk", 1, 1, 2)],
                    9: [("qk", 1, 1, 3)],
                    12: [("qk", 0, 0, 1)],
                },
                2: {  # (0,1)
                    2: [("qk", 1, 0, 1)],
                    6: [("qk", 0, 0, 2)],
                },
                3: {2: [("qk", 1, 0, 2)]},  # (1,1)
                4: {2: [("qk", 0, 0, 3)]},  # (0,2)
                5: {2: [("qk", 1, 0, 3)]},  # (1,2)
                6: {},  # (0,3)
                7: {},  # (1,3)
            }
            # vT[s] is read by ctx at phase body s+2; its bias lands at
            # emit-body+1 (before that body's events) => emit at body <= s+1
            for _j, _specs in il[0].items():
                for _sp in _specs:
                    if _sp[0] == "v":
                        assert _j <= _sp[1] + 1, (_j, _sp)

            # --- attention: flat 128-body stream with an event schedule ---
            # events[g] = closures to emit at global body g (after scores+exp)
            events = {}

            def add_event(g, fn, front=False):
                lst = events.setdefault(g, [])
                if front:
                    lst.insert(0, fn)
                else:
                    lst.append(fn)

            cx_of = {}  # phase index -> [cx_hh0, cx_hh1]
            pt_of = {}  # (phase index, pair m) -> pt pair tile

            def mk_mask(ph, m, jp):
                # ONE FD=1024 bf16 2x op per (pair, j): operates on the
                # contiguous [2(hh), 512] block of one j-tile, with the mask
                # row broadcast over hh (stride-0). Gated on only THAT
                # j-tile's exp, so it runs during the next exp instead of
                # serializing after the pair's second exp.
                p, blk = PHASES[ph]

                def fn():
                    pt = pt_of[(ph, m)]
                    nc.vector.tensor_mul(
                        pt[:, jp, :, :],
                        pt[:, jp, :, :],
                        maskT[:, 2 * m + jp, bass.ds(blk * 512, 512)]
                        .unsqueeze(1)
                        .broadcast_to([128, 2, 512]),
                    )

                return fn

            def mk_ctx(ph, m, jp):
                # one j-tile's ctx (2 MMs) per body — keeps the PE FIFO
                # load uniform so the scores->exp chain never overruns the
                # ACT pace
                p, blk = PHASES[ph]

                def fn():
                    pt = pt_of[(ph, m)]
                    if jp == 1:
                        del pt_of[(ph, m)]
                    if ph not in cx_of:
                        # first ctx write of the phase: allocate cx AFTER
                        # the previous phase's drain is emitted (the drain
                        # is front-inserted in this body's event list)
                        cx_of[ph] = [
                            cxp.tile([65, 512], F32, tag="cx", name=f"cx{ph}{i}")
                            for i in range(2)
                        ]
                    cxt = cx_of[ph]
                    jj = 2 * m + jp
                    for hh in range(2):
                        hloc = 2 * p + hh
                        nc.tensor.matmul(
                            cxt[hh][:],
                            lhsT=vT[:, jj, bass.ds(hloc * 65, 65)],
                            rhs=pt[:, jp, hh, :],
                            start=(jj == 0),
                            stop=(jj == 15),
                        )

                return fn

            def mk_drain(ph):
                p, blk = PHASES[ph]

                def fn():
                    cxt = cx_of.pop(ph)
                    for hh in range(2):
                        hloc = 2 * p + hh
                        ob = obp.tile([65, 512], BF, tag="ob")
                        nc.vector.tensor_copy(ob[:], cxt[hh][:])
                        nc.sync.dma_start(
                            Od[bass.ds(hloc * 65, 65), bass.ds(blk * 512, 512)],
                            ob[:],
                        )

                return fn

            for ph in range(8):
                base = 16 * ph
                for m in range(8):
                    add_event(base + 2 * m + 1, mk_mask(ph, m, 0))
                    add_event(base + 2 * m + 2, mk_mask(ph, m, 1))
                    add_event(base + 2 * m + 2, mk_ctx(ph, m, 0))
                    add_event(base + 2 * m + 3, mk_ctx(ph, m, 1))
                # drain one body after the last ctx; at the list FRONT so
                # the next phase's first ctx (same body) still sees the WAR
                # on the cx banks
                add_event(base + 18, mk_drain(ph), front=True)

            pend_bias = []
            for g in range(16 * 8 + 4):
                if g < 16 * 8:
                    ph = g // 16
                    p, blk = PHASES[ph]
                    j = g % 16
                    # deferred DVE bias moves from the previous body's chunks
                    for fn in pend_bias:
                        fn()
                    pend_bias = []
                    # scores: one sc tile, two co-issued matmuls, one exp
                    sc = scp.tile([128, 2, 512], F32, tag="sc")
                    for hh in range(2):
                        lo, hi = 64 * hh, 64 * hh + 64
                        nc.tensor.matmul(
                            sc[:, hh, :],
                            lhsT=k_sb[lo:hi, p, bass.ts(j, 128)],
                            rhs=q_sb[lo:hi, p, bass.ts(blk, 512)],
                            start=True,
                            stop=True,
                        )
                    if j % 2 == 0:
                        ptpair = ptp.tile([128, 2, 2, 512], BF, tag="pt")
                        pt_of[(ph, j // 2)] = ptpair
                    else:
                        ptpair = pt_of[(ph, j // 2)]
                    nc.scalar.activation(ptpair[:, j % 2, :, :], sc[:], EXP)
                # scheduled events (masks / lagged ctx / drains)
                for fn in events.pop(g, ()):
                    fn()
                # projection chunks last on the PE FIFO
                if g < 16 * 8:
                    if ph == 0 and j == 4:
                        refill_v(0)
                    elif ph == 0 and j == 8:
                        refill_v(1)
                    for spec in il[ph].get(j, ()):
                        if spec[0] == "v":
                            pend_bias.append(proj_v(spec[1]))
                        else:
                            pend_bias.append(proj_qk(*spec[1:]))
            for fn in pend_bias:
                fn()
    nc.compile()
    return nc


def _get_nc():
    if "nc" not in _NC_CACHE:
        _NC_CACHE["nc"] = build_nc()
    return _NC_CACHE["nc"]


def _make_in_maps(Q, K, V, mask, Wq, bq, Wk, bk, Wv, bv):
    per_batch = []
    for b in range(B):
        Qa = Q[b].astype(NBF)
        Ka = K[b].astype(NBF)
        Va = V[b].astype(NBF)
        mT = np.ascontiguousarray((~mask[b]).T).astype(np.float32).astype(NBF)
        per_batch.append((Qa, Ka, Va, mT))

    in_maps = []
    for c in range(N_CORES):
        b, g = divmod(c, 4)
        hs = slice(g * CPC, (g + 1) * CPC)
        Qa, Ka, Va, mT = per_batch[b]
        # pre-arranged to the SBUF layout [128, ci, n] so the weight DMA
        # is a single contiguous transfer
        WqTa = np.ascontiguousarray(
            (Wq[hs].T / 8.0).reshape(8, 128, CPC).transpose(1, 0, 2).reshape(128, -1)
        ).astype(NBF)
        WkTa = np.ascontiguousarray(
            Wk[hs].T.reshape(8, 128, CPC).transpose(1, 0, 2).reshape(128, -1)
        ).astype(NBF)
        WvTa = np.zeros((C, HPC * 65), np.float32)
        bvba = np.zeros((128, HPC * 65), np.float32)
        for hh in range(HPC):
            ch = slice((g * HPC + hh) * CPH, (g * HPC + hh + 1) * CPH)
            WvTa[:, hh * 65 : hh * 65 + 64] = Wv[ch].T
            bvba[:, hh * 65 : hh * 65 + 64] = bv[ch][None, :]
            bvba[:, hh * 65 + 64] = 1.0
        # bias for q/k psum->sbuf copies: col 2p+qk = per-partition bias of
        # pair p's 128 channels (rows 0:64 = head 2p, 64:128 = head 2p+1)
        bqka = np.zeros((128, 4), np.float32)
        for p in range(2):
            ch = slice((g * 2 + p) * 128, (g * 2 + p + 1) * 128)
            bqka[:, 2 * p] = bq[ch] / 8.0
            bqka[:, 2 * p + 1] = bk[ch]
        in_maps.append(
            {
                "Qin": Qa,
                "Kin": Ka,
                "Vin": Va,
                "WqT": WqTa,
                "WkT": WkTa,
                "WvT": np.ascontiguousarray(
                    WvTa.reshape(8, 128, HPC * 65)
                    .transpose(1, 0, 2)
                    .reshape(128, -1)
                ).astype(NBF),
                "bqk": bqka,
                "bvb": bvba,
                "maskT": mT,
            }
        )
    return in_maps


def _assemble(results):
    out = np.zeros((B, S, C), np.float32)
    for c in range(N_CORES):
        b, g = divmod(c, 4)
        o = results[c]["out"].astype(np.float32)  # [260, 2048]
        for hh in range(HPC):
            ctx = o[hh * 65 : hh * 65 + 64]  # [64, S] = (d, i)
            den = o[hh * 65 + 64]  # [S]
            ch0 = (g * HPC + hh) * CPH
            out[b, :, ch0 : ch0 + CPH] = (ctx / den[None, :]).T
    return out


def run(inputs, trace=False):
    in_maps = _make_in_maps(
        np.asarray(inputs["Q"], np.float32),
        np.asarray(inputs["K"], np.float32),
        np.asarray(inputs["V"], np.float32),
        np.asarray(inputs["mask"]),
        np.asarray(inputs["Wq"], np.float32),
        np.asarray(inputs["bq"], np.float32),
        np.asarray(inputs["Wk"], np.float32),
        np.asarray(inputs["bk"], np.float32),
        np.asarray(inputs["Wv"], np.float32),
        np.asarray(inputs["bv"], np.float32),
    )
    br = run_bass_kernel_spmd(_get_nc(), in_maps, list(range(N_CORES)), trace=trace)
    return _assemble(br.results), br


def kernel(**inputs) -> np.ndarray:
    out, _ = run(inputs)
    return out


# revision 34
# speedup vs baseline: 1.0487x; 1.0041x over previous
"""Trainium2 Bass kernel for nn_MultiHeadAttention (B=2, C=1024, H=16, S=2048).

Sharding: 8 cores = 2 batches x 4 head-groups (4 heads per core).

The kernel is ACT(exp)-paced: 128 exps of [128, 1024] PSUM->SBUF (~1.0us
back-to-back) =~ 128us floor; everything else is scheduled to hide under it.

Structure (per core):
  - 8 phases (p, blk): head-pair p x 512-query block blk, ordered
    (0,0),(1,0),(0,1),(1,1),... so the odd phases (pair 1) need no new DMA
    and the input stream hides behind them.
  - Per body (one j-tile): ONE sc tile [128, 2(hh), 512] (2 PSUM banks),
    2 score matmuls that co-issue on disjoint PE row groups (partition
    halves 0:64/64:128), ONE exp of FD=1024.
  - exp writes into a pt PAIR tile [128, 2(j), 2(hh), 512]; the mask
    multiply runs once per (pair, hh) as an FD=1024 bf16 2x DVE op with a
    strided [2, 512] access pattern over two j-rows of maskT.
  - ctx matmuls for a j-pair are emitted 3 bodies later (after the mask is
    guaranteed done) so the PE FIFO never head-of-line blocks ahead of the
    next scores. ctx uses the 65-col ones trick for the softmax denominator.
  - PSUM: 2 score slots (2 banks each) + 2 ctx accumulators (1 bank each)
    + a DEDICATED 2-slot projection pool (1 bank each) = 8 banks. The
    projection chunks never share slots with scores, so their PSUM->SBUF
    bias moves (DVE, deferred one body) are fully elastic.
  - DMA: one instruction per 512-column group via a (x p) n rearrange
    (8 DRAM row-blocks per instruction), masks two j-rows at a time;
    need-ordered so phase (0,0) is gated only by W+K+V+Q[0:512]+mask-q0.
  - Host does the final divide by the denominator row + transpose/concat.
"""

import numpy as np
import ml_dtypes

import concourse.bass as bass
import concourse.mybir as mybir
import concourse.tile as tile
from concourse import bacc
from concourse.bass_utils import run_bass_kernel_spmd

B = 2
C = 1024
HEADS = 16
CPH = 64
S = 2048
N_CORES = 8
HPC = 4  # heads per core
CPC = HPC * CPH  # channels per core = 256

BF = mybir.dt.bfloat16
F32 = mybir.dt.float32
EXP = mybir.ActivationFunctionType.Exp

NBF = ml_dtypes.bfloat16

_NC_CACHE = {}

# phase order: (p, blk)
PHASES = [(0, 0), (1, 0), (0, 1), (1, 1), (0, 2), (1, 2), (0, 3), (1, 3)]


def build_nc():
    nc = bacc.Bacc("TRN2", target_bir_lowering=False)

    Qd = nc.declare_dram_parameter("Qin", [C, S], BF, isOutput=False)
    Kd = nc.declare_dram_parameter("Kin", [C, S], BF, isOutput=False)
    Vd = nc.declare_dram_parameter("Vin", [C, S], BF, isOutput=False)
    WqTd = nc.declare_dram_parameter("WqT", [128, 8 * CPC], BF, isOutput=False)
    WkTd = nc.declare_dram_parameter("WkT", [128, 8 * CPC], BF, isOutput=False)
    WvTd = nc.declare_dram_parameter("WvT", [128, 8 * HPC * 65], BF, isOutput=False)
    bqkd = nc.declare_dram_parameter("bqk", [128, 4], F32, isOutput=False)
    bvbd = nc.declare_dram_parameter("bvb", [128, HPC * 65], F32, isOutput=False)
    Md = nc.declare_dram_parameter("maskT", [S, S], BF, isOutput=False)
    Od = nc.declare_dram_parameter("out", [HPC * 65, S], BF, isOutput=True)

    with tile.TileContext(nc) as tc:
        with (
            tc.tile_pool(name="w", bufs=1) as wp,
            tc.tile_pool(name="qksb", bufs=1) as qkp,
            tc.tile_pool(name="vt", bufs=1) as vtp,
            tc.tile_pool(name="msk", bufs=1) as mkp,
            tc.tile_pool(name="ioqk", bufs=1) as ioqk,
            tc.tile_pool(name="pt", bufs=3) as ptp,
            tc.tile_pool(name="ob", bufs=2) as obp,
            tc.tile_pool(name="sc", bufs=2, space="PSUM") as scp,
            tc.tile_pool(name="cx", bufs=2, space="PSUM") as cxp,
            tc.tile_pool(name="prj", bufs=2, space="PSUM") as prjp,
        ):
            # --- persistent SBUF tensors ---
            WqT = wp.tile([128, 8, CPC], BF, tag="wq")
            WkT = wp.tile([128, 8, CPC], BF, tag="wk")
            WvT = wp.tile([128, 8, HPC * 65], BF, tag="wv")
            bqk = wp.tile([128, 4], F32, tag="bqk")
            bvb = wp.tile([128, HPC * 65], F32, tag="bvb")
            dummy = wp.tile([128, 1], F32, tag="dum")

            q_sb = qkp.tile([128, 2, S], BF, tag="q")  # pair-major, head rows 0:64/64:128
            k_sb = qkp.tile([128, 2, S], BF, tag="k")
            vT = vtp.tile([128, 16, HPC * 65], BF, tag="vt")  # s_tile-major
            maskT = mkp.tile([128, 16, S], BF, tag="m")
            Qin = ioqk.tile([128, 8, S], BF, tag="qi")
            Kin = ioqk.tile([128, 8, S], BF, tag="ki")
            # V is consumed early (phase (0,0)); half-size buffer, h1 is
            # re-DMA'd over h0 via the idle GPSIMD queue (no head-of-line
            # blocking of the main sync DMA stream)
            Vin = ioqk.tile([128, 8, 1024], BF, tag="vi")

            # --- DMA emitters; emission order sets arrival order ---
            def dma_cols(buf, dram, c0, w):
                # one instruction: all 8 row-blocks of a column group
                nc.sync.dma_start(
                    buf[:, :, bass.ds(c0, w)],
                    dram[:, bass.ds(c0, w)].rearrange("(x p) n -> p x n", x=8),
                )

            def dma_m2(j2, c0, w):
                # two j-rows (256 DRAM rows) per instruction
                nc.sync.dma_start(
                    maskT[:, 2 * j2 : 2 * j2 + 2, bass.ds(c0, w)],
                    Md[bass.ds(256 * j2, 256), bass.ds(c0, w)].rearrange(
                        "(x p) n -> p x n", x=2
                    ),
                )

            # weights first (needed by the upfront projections)
            for wt, wd in ((WvT, WvTd), (WkT, WkTd), (WqT, WqTd)):
                nc.sync.dma_start(wt[:], wd[:].rearrange("p (t n) -> p t n", t=8))
            nc.sync.dma_start(bqk[:], bqkd[:])
            nc.sync.dma_start(bvb[:], bvbd[:])
            # absorb the exp ACT_TABLE_LOAD (~2.7us) during startup
            nc.scalar.activation(dummy[:], bqk[:, 0:1], EXP)

            # need-ordered input stream. Phase (0,0) is gated by:
            # W + K(full) + V(full) + Q[0:512] + mask q0 ~ 12.7 MB.
            dma_cols(Kin, Kd, 0, 512)
            dma_cols(Qin, Qd, 0, 512)
            dma_cols(Vin, Vd, 0, 512)
            for j2 in range(2):
                dma_m2(j2, 0, 512)
            dma_cols(Kin, Kd, 512, 512)
            for j2 in range(2, 4):
                dma_m2(j2, 0, 512)
            dma_cols(Vin, Vd, 512, 512)
            for j2 in range(4, 6):
                dma_m2(j2, 0, 512)
            dma_cols(Kin, Kd, 1024, 512)
            dma_m2(6, 0, 512)
            dma_cols(Kin, Kd, 1536, 512)
            dma_m2(7, 0, 512)
            dma_cols(Qin, Qd, 512, 512)
            for j2 in range(8):  # mask q1 (for phases blk=1)
                dma_m2(j2, 512, 512)
            dma_cols(Qin, Qd, 1024, 512)
            for j2 in range(8):  # mask h1 (for phases blk=2,3)
                dma_m2(j2, 1024, 1024)
            dma_cols(Qin, Qd, 1536, 512)

            # --- projection chunks: own 1-bank PSUM slots, deferred bias ---
            def proj_v(s):
                ps = prjp.tile([128, 512], F32, tag="prj", name="pv")
                for ci in range(8):
                    nc.tensor.matmul(
                        ps[:, : HPC * 65],
                        lhsT=Vin[:, ci, bass.ts(s % 8, 128)],
                        rhs=WvT[:, ci, :],
                        start=(ci == 0),
                        stop=(ci == 7),
                    )

                def bias():
                    nc.vector.tensor_add(vT[:, s, :], ps[:, : HPC * 65], bvb[:])

                return bias

            def proj_qk(p, qk, n4):
                dst, wt, src = (
                    (q_sb, WqT, Qin) if qk == 0 else (k_sb, WkT, Kin)
                )
                ps = prjp.tile([128, 512], F32, tag="prj", name="pqk")
                for ci in range(8):
                    nc.tensor.matmul(
                        ps[:],
                        lhsT=wt[:, ci, bass.ts(p, 128)],
                        rhs=src[:, ci, bass.ts(n4, 512)],
                        start=(ci == 0),
                        stop=(ci == 7),
                    )

                def bias():
                    nc.vector.tensor_scalar_add(
                        dst[:, p, bass.ts(n4, 512)],
                        ps[:],
                        bqk[:, 2 * p + qk : 2 * p + qk + 1],
                    )

                return bias

            # upfront projections: only what scores_0 needs — everything
            # else goes into loop bodies behind the scores on the PE FIFO
            proj_qk(0, 1, 0)()
            proj_qk(0, 0, 0)()

            def refill_v(half):
                # V h1 re-DMA over the consumed h0 half-buffer; GPSIMD
                # queue so the WAR wait can't head-of-line block the main
                # input stream. Emitted AFTER the last h0 readers so the
                # dependency is visible to the tile framework.
                nc.gpsimd.dma_start(
                    Vin[:, :, bass.ts(half, 512)],
                    Vd[:, bass.ds(1024 + half * 512, 512)].rearrange(
                        "(x p) n -> p x n", x=8
                    ),
                )

            # in-loop projection schedule: phase index -> {body j: [chunks]}
            il = {
                0: {  # (0,0) — DMA-gated phase; PE slack absorbs the chunks
                    0: [("v", 0), ("v", 1)],
                    1: [("v", 2), ("qk", 0, 1, 1)],
                    2: [("v", 3), ("v", 4)],
                    3: [("v", 5)],
                    5: [("qk", 0, 1, 2)],
                    6: [("v", 6)],
                    7: [("v", 7)],
                    9: [("v", 8), ("v", 9)],
                    10: [("qk", 0, 1, 3)],
                    11: [("v", 10)],
                    12: [("v", 11)],
                    13: [("v", 12), ("qk", 1, 1, 0)],
                    14: [("v", 13), ("qk", 1, 0, 0)],
                    15: [("v", 14), ("v", 15)],
                },
                1: {  # (1,0)
                    1: [("qk", 1, 1, 1)],
                    5: [("qk", 1, 1, 2)],
                    9: [("qk", 1, 1, 3)],
                    12: [("qk", 0, 0, 1)],
                },
                2: {  # (0,1)
                    2: [("qk", 1, 0, 1)],
                    6: [("qk", 0, 0, 2)],
                },
                3: {2: [("qk", 1, 0, 2)]},  # (1,1)
                4: {2: [("qk", 0, 0, 3)]},  # (0,2)
                5: {2: [("qk", 1, 0, 3)]},  # (1,2)
                6: {},  # (0,3)
                7: {},  # (1,3)
            }
            # vT[s] is read by ctx at phase body s+2; its bias lands at
            # emit-body+1 (before that body's events) => emit at body <= s+1
            for _j, _specs in il[0].items():
                for _sp in _specs:
                    if _sp[0] == "v":
                        assert _j <= _sp[1] + 1, (_j, _sp)

            # --- attention: flat 128-body stream with an event schedule ---
            # events[g] = closures to emit at global body g (after scores+exp)
            events = {}

            def add_event(g, fn, prio=0):
                # prio orders closures within a body: masks (0) before the
                # drain (1) before ctx (2) — the drain's DVE copies must not
                # sit ahead of the next phase's mask on the DVE FIFO, and
                # the next phase's first ctx must still be emitted after the
                # drain so the WAR on the cx banks is visible
                events.setdefault(g, []).append((prio, len(events.get(g, ())), fn))

            cx_of = {}  # phase index -> [cx_hh0, cx_hh1]
            pt_of = {}  # (phase index, pair m) -> pt pair tile

            def mk_mask(ph, m, jp):
                # ONE FD=1024 bf16 2x op per (pair, j): operates on the
                # contiguous [2(hh), 512] block of one j-tile, with the mask
                # row broadcast over hh (stride-0). Gated on only THAT
                # j-tile's exp, so it runs during the next exp instead of
                # serializing after the pair's second exp.
                p, blk = PHASES[ph]

                def fn():
                    pt = pt_of[(ph, m)]
                    nc.vector.tensor_mul(
                        pt[:, jp, :, :],
                        pt[:, jp, :, :],
                        maskT[:, 2 * m + jp, bass.ds(blk * 512, 512)]
                        .unsqueeze(1)
                        .broadcast_to([128, 2, 512]),
                    )

                return fn

            def mk_ctx(ph, m, jp):
                # one j-tile's ctx (2 MMs) per body — keeps the PE FIFO
                # load uniform so the scores->exp chain never overruns the
                # ACT pace
                p, blk = PHASES[ph]

                def fn():
                    pt = pt_of[(ph, m)]
                    if jp == 1:
                        del pt_of[(ph, m)]
                    if ph not in cx_of:
                        # first ctx write of the phase: allocate cx AFTER
                        # the previous phase's drain is emitted (the drain
                        # is front-inserted in this body's event list)
                        cx_of[ph] = [
                            cxp.tile([65, 512], F32, tag="cx", name=f"cx{ph}{i}")
                            for i in range(2)
                        ]
                    cxt = cx_of[ph]
                    jj = 2 * m + jp
                    for hh in range(2):
                        hloc = 2 * p + hh
                        nc.tensor.matmul(
                            cxt[hh][:],
                            lhsT=vT[:, jj, bass.ds(hloc * 65, 65)],
                            rhs=pt[:, jp, hh, :],
                            start=(jj == 0),
                            stop=(jj == 15),
                        )

                return fn

            def mk_drain(ph):
                p, blk = PHASES[ph]

                def fn():
                    cxt = cx_of.pop(ph)
                    for hh in range(2):
                        hloc = 2 * p + hh
                        ob = obp.tile([65, 512], BF, tag="ob")
                        nc.vector.tensor_copy(ob[:], cxt[hh][:])
                        # gpsimd DMA queue: the copy-gated output DMA must
                        # not head-of-line block the sync input stream
                        nc.gpsimd.dma_start(
                            Od[bass.ds(hloc * 65, 65), bass.ds(blk * 512, 512)],
                            ob[:],
                        )

                return fn

            for ph in range(8):
                base = 16 * ph
                for m in range(8):
                    add_event(base + 2 * m + 1, mk_mask(ph, m, 0))
                    add_event(base + 2 * m + 2, mk_mask(ph, m, 1))
                    add_event(base + 2 * m + 2, mk_ctx(ph, m, 0), prio=2)
                    add_event(base + 2 * m + 3, mk_ctx(ph, m, 1), prio=2)
                add_event(base + 18, mk_drain(ph), prio=1)

            pend_bias = []
            for g in range(16 * 8 + 4):
                if g < 16 * 8:
                    ph = g // 16
                    p, blk = PHASES[ph]
                    j = g % 16
                    # deferred DVE bias moves from the previous body's chunks
                    for fn in pend_bias:
                        fn()
                    pend_bias = []
                    # scores: one sc tile, two co-issued matmuls, one exp
                    sc = scp.tile([128, 2, 512], F32, tag="sc")
                    for hh in range(2):
                        lo, hi = 64 * hh, 64 * hh + 64
                        nc.tensor.matmul(
                            sc[:, hh, :],
                            lhsT=k_sb[lo:hi, p, bass.ts(j, 128)],
                            rhs=q_sb[lo:hi, p, bass.ts(blk, 512)],
                            start=True,
                            stop=True,
                        )
                    if j % 2 == 0:
                        ptpair = ptp.tile([128, 2, 2, 512], BF, tag="pt")
                        pt_of[(ph, j // 2)] = ptpair
                    else:
                        ptpair = pt_of[(ph, j // 2)]
                    nc.scalar.activation(ptpair[:, j % 2, :, :], sc[:], EXP)
                # scheduled events (masks / lagged ctx / drains)
                for _, _, fn in sorted(events.pop(g, []))[: None]:
                    fn()
                # projection chunks last on the PE FIFO
                if g < 16 * 8:
                    if ph == 0 and j == 4:
                        refill_v(0)
                    elif ph == 0 and j == 8:
                        refill_v(1)
                    for spec in il[ph].get(j, ()):
                        if spec[0] == "v":
                            pend_bias.append(proj_v(spec[1]))
                        else:
                            pend_bias.append(proj_qk(*spec[1:]))
            for fn in pend_bias:
                fn()
    nc.compile()
    return nc


def _get_nc():
    if "nc" not in _NC_CACHE:
        _NC_CACHE["nc"] = build_nc()
    return _NC_CACHE["nc"]


def _make_in_maps(Q, K, V, mask, Wq, bq, Wk, bk, Wv, bv):
    per_batch = []
    for b in range(B):
        Qa = Q[b].astype(NBF)
        Ka = K[b].astype(NBF)
        Va = V[b].astype(NBF)
        mT = np.ascontiguousarray((~mask[b]).T).astype(np.float32).astype(NBF)
        per_batch.append((Qa, Ka, Va, mT))

    in_maps = []
    for c in range(N_CORES):
        b, g = divmod(c, 4)
        hs = slice(g * CPC, (g + 1) * CPC)
        Qa, Ka, Va, mT = per_batch[b]
        # pre-arranged to the SBUF layout [128, ci, n] so the weight DMA
        # is a single contiguous transfer
        WqTa = np.ascontiguousarray(
            (Wq[hs].T / 8.0).reshape(8, 128, CPC).transpose(1, 0, 2).reshape(128, -1)
        ).astype(NBF)
        WkTa = np.ascontiguousarray(
            Wk[hs].T.reshape(8, 128, CPC).transpose(1, 0, 2).reshape(128, -1)
        ).astype(NBF)
        WvTa = np.zeros((C, HPC * 65), np.float32)
        bvba = np.zeros((128, HPC * 65), np.float32)
        for hh in range(HPC):
            ch = slice((g * HPC + hh) * CPH, (g * HPC + hh + 1) * CPH)
            WvTa[:, hh * 65 : hh * 65 + 64] = Wv[ch].T
            bvba[:, hh * 65 : hh * 65 + 64] = bv[ch][None, :]
            bvba[:, hh * 65 + 64] = 1.0
        # bias for q/k psum->sbuf copies: col 2p+qk = per-partition bias of
        # pair p's 128 channels (rows 0:64 = head 2p, 64:128 = head 2p+1)
        bqka = np.zeros((128, 4), np.float32)
        for p in range(2):
            ch = slice((g * 2 + p) * 128, (g * 2 + p + 1) * 128)
            bqka[:, 2 * p] = bq[ch] / 8.0
            bqka[:, 2 * p + 1] = bk[ch]
        in_maps.append(
            {
                "Qin": Qa,
                "Kin": Ka,
                "Vin": Va,
                "WqT": WqTa,
                "WkT": WkTa,
                "WvT": np.ascontiguousarray(
                    WvTa.reshape(8, 128, HPC * 65)
                    .transpose(1, 0, 2)
                    .reshape(128, -1)
                ).astype(NBF),
                "bqk": bqka,
                "bvb": bvba,
                "maskT": mT,
            }
        )
    return in_maps


def _assemble(results):
    out = np.zeros((B, S, C), np.float32)
    for c in range(N_CORES):
        b, g = divmod(c, 4)
        o = results[c]["out"].astype(np.float32)  # [260, 2048]
        for hh in range(HPC):
            ctx = o[hh * 65 : hh * 65 + 64]  # [64, S] = (d, i)
            den = o[hh * 65 + 64]  # [S]
            ch0 = (g * HPC + hh) * CPH
            out[b, :, ch0 : ch0 + CPH] = (ctx / den[None, :]).T
    return out


def run(inputs, trace=False):
    in_maps = _make_in_maps(
        np.asarray(inputs["Q"], np.float32),
        np.asarray(inputs["K"], np.float32),
        np.asarray(inputs["V"], np.float32),
        np.asarray(inputs["mask"]),
        np.asarray(inputs["Wq"], np.float32),
        np.asarray(inputs["bq"], np.float32),
        np.asarray(inputs["Wk"], np.float32),
        np.asarray(inputs["bk"], np.float32),
        np.asarray(inputs["Wv"], np.float32),
        np.asarray(inputs["bv"], np.float32),
    )
    br = run_bass_kernel_spmd(_get_nc(), in_maps, list(range(N_CORES)), trace=trace)
    return _assemble(br.results), br


def kernel(**inputs) -> np.ndarray:
    out, _ = run(inputs)
    return out
